# revision 36
# baseline (speedup 1.0000x reference)
"""Fused QKV + RMSNorm + RoPE + self-attention kernel for Trainium2.

Sharding: tensor-parallel over heads. 16 heads / 8 cores = 2 heads per core.
Each core computes the qkv projection for its 2 heads (column-parallel),
per-head RMSNorm/RoPE/attention locally, and exports an UNNORMALIZED
attention output [d, q] plus partial softmax denominators; the host divides,
transposes to token-major and concatenates the head slices (the output
projection is absent, so the all-gather is a host-side concat).

Structural notes:
  - the projection runs as two half-passes per batch (head-0 columns, then
    head-1) so attention for a (batch, head) starts right after its
    half-pass; the ACT-bound attention tail is only 4 qgroups.
  - softmax exp batched into N=1024 activation instructions (pipelines at
    ~996ns on ScalarE); denominators exported as folded partial sums;
    normalization/transpose/concat on host.
  - rsqrt via bit-trick + 1 Newton step, batched [P,8] per 4-tile group.
  - per-group transposes collected in one PSUM bank, one DVE evacuation.
  - host-side input layouts give 2-4KB contiguous DMA runs per partition.

Compute dtype: bf16 matmuls with fp32 accumulation; stats fp32.
"""

import sys

sys.path.insert(0, "/opt/trn_rl_repo")

import numpy as np
import ml_dtypes

import concourse.bass as bass
import concourse.mybir as mybir
import concourse.tile as tile
from concourse import bacc
from concourse.masks import make_identity

B = 2
SEQ = 2048
DIM = 2048
NHEADS = 16
HEAD_DIM = 128
NCORES = 8
HPC = NHEADS // NCORES  # heads per core = 2
EPS = 1e-6
SCALE = float(HEAD_DIM) ** -0.5
P = 128

F32 = mybir.dt.float32
BF16 = mybir.dt.bfloat16
I32 = mybir.dt.int32

QG = 512  # q tokens per attention group
GT = 4  # tiles per p1 group (rsqrt-chain batch)


def build_nc(seq=SEQ, batches=B):
    ntb = seq // P  # token tiles per batch (16)
    kc_n = DIM // P  # contraction chunks (16)
    HF = 3 * HEAD_DIM  # features per head = 384 (q,k,v)
    qg_per = seq // QG  # q groups per (b, h) = 4
    gq = QG // P  # 128-tiles per q group (4)
    ngrp = ntb // GT  # p1 groups per batch (4)

    nc = bacc.Bacc(None, target_bir_lowering=False)

    xt_ext = nc.declare_dram_parameter(
        "xt", [P, batches * ntb, kc_n, P], BF16, isOutput=False
    )
    wt_ext = nc.declare_dram_parameter("wt", [P, kc_n, 2 * HF], BF16, isOutput=False)
    bias_ext = nc.declare_dram_parameter("bias", [1, 2 * HF], F32, isOutput=False)
    csc_ext = nc.declare_dram_parameter("csc", [P, ntb, 2 * P], BF16, isOutput=False)
    av_ext = nc.declare_dram_parameter(
        "av", [batches, HPC, P, seq], F32, isOutput=True
    )
    sums_ext = nc.declare_dram_parameter(
        "sums", [batches, HPC, qg_per, P, 2 * QG], BF16, isOutput=True
    )

    add = mybir.AluOpType.add
    sub = mybir.AluOpType.subtract
    mul = mybir.AluOpType.mult

    with tile.TileContext(nc) as tc:
        with (
            tc.tile_pool(name="consts", bufs=1) as consts,
            tc.tile_pool(name="persist", bufs=1) as persist,
        ):
            ident = consts.tile([P, P], BF16, tag="ident")
            make_identity(nc, ident[:])

            # first x tiles are the critical path at startup: their DMAs
            # are issued first, interleaved with the wt chunks they need
            x_pre = []
            wt_sb = consts.tile([P, kc_n, 2 * HF], BF16, tag="wt")
            for t in range(2):
                xp = consts.tile([P, kc_n, P], BF16, tag=f"xpre{t}", name="xp")
                # 16 parallel chunk DMAs: the first tiles land in a few us
                # instead of one long single-queue transfer
                for kc in range(kc_n):
                    nc.sync.dma_start(
                        out=xp[:, kc : kc + 1, :],
                        in_=xt_ext[:, t, kc : kc + 1, :],
                    )
                x_pre.append(xp)
            nc.sync.dma_start(out=wt_sb[:, 0:2, :], in_=wt_ext[:, 0:2, :])
            bias_sb = consts.tile([P, 2 * HF], F32, tag="bias")
            bap = bias_ext[:]
            bias_bcast = bass.AP(
                tensor=bap.tensor, offset=bap.offset, ap=[[0, P], [1, 2 * HF]]
            )
            nc.sync.dma_start(out=bias_sb[:], in_=bias_bcast)
            for kq in range(1, 8):
                nc.sync.dma_start(
                    out=wt_sb[:, 2 * kq : 2 * kq + 2, :],
                    in_=wt_ext[:, 2 * kq : 2 * kq + 2, :],
                )
            csc_sb = consts.tile([P, ntb, 2 * P], BF16, tag="csc")
            nc.sync.dma_start(out=csc_sb[:], in_=csc_ext[:])

            # persistent per-(batch, head) operands: qk feature-major
            # [d, {q,k}, tile, tok]; v token-major [tok, tile, d]
            qk = {}
            vv = {}
            for b in range(batches):
                for hl in range(HPC):
                    qk[(b, hl)] = persist.tile(
                        [P, 2, ntb, P], BF16, tag=f"qk{b}_{hl}", name=f"qk{b}_{hl}"
                    )
                    vv[(b, hl)] = persist.tile(
                        [P, ntb, P], BF16, tag=f"v{b}_{hl}", name=f"v{b}_{hl}"
                    )

            with (
                tc.tile_pool(name="p1", bufs=2) as p1,
                tc.tile_pool(name="p1x", bufs=6) as p1x,
                tc.tile_pool(name="p1s", bufs=2) as p1s,
                tc.tile_pool(name="p2", bufs=2) as p2,
                tc.tile_pool(name="psp", bufs=1, space="PSUM") as psp,
            ):

                def stage_a(b, g, hl, ms8):
                    """Projection half-pass for head hl, tiles g*GT..+GT.
                    Writes qkv_sb tiles (bf16) and ms8 [P, 8] sumsq stats.
                    Returns the list of qkv_sb tiles."""
                    qkvs = []
                    c0 = hl * HF
                    for u in range(GT):
                        ti = g * GT + u
                        t = b * ntb + ti
                        if b == 0 and hl == 0 and ti < len(x_pre):
                            x_tile = x_pre[ti]
                        else:
                            x_tile = p1x.tile(
                                [P, kc_n, P], BF16, tag="x", name="x"
                            )
                            nc.sync.dma_start(
                                out=x_tile[:], in_=xt_ext[:, t, :, :]
                            )
                        ps = psp.tile([P, HF], F32, tag="ps", bufs=2, name="ps")
                        for kc in range(kc_n):
                            nc.tensor.matmul(
                                ps[:],
                                x_tile[:, kc, :],
                                wt_sb[:, kc, c0 : c0 + HF],
                                start=(kc == 0),
                                stop=(kc == kc_n - 1),
                            )
                        qkv_sb = p1.tile([P, HF], BF16, tag="qkv", bufs=2 * GT)
                        nc.vector.tensor_tensor(
                            qkv_sb[:], ps[:], bias_sb[:, c0 : c0 + HF], add
                        )
                        for blk in range(2):
                            xb = qkv_sb[:, blk * P : (blk + 1) * P]
                            acc = ms8[:, 2 * u + blk : 2 * u + blk + 1]
                            if b == 0:
                                sq = p1s.tile([P, P], BF16, tag="sqa")
                                nc.scalar.activation(
                                    out=sq[:],
                                    in_=xb,
                                    func=mybir.ActivationFunctionType.Square,
                                    accum_out=acc,
                                )
                            else:
                                sq = p1s.tile([P, P], BF16, tag="sqv")
                                nc.vector.scalar_tensor_tensor(
                                    sq[:], xb, 1.0, xb, mul, mul, accum_out=acc
                                )
                        qkvs.append(qkv_sb)
                    return qkvs

                def stage_b(b, g, hl, ms8, qkvs):
                    """rsqrt chain for the group, then rope + transpose +
                    v-copy per tile."""
                    aa = p1s.tile([P, 2 * GT], F32, tag="aa")
                    nc.vector.tensor_scalar(
                        aa[:], ms8[:], 1.0 / HEAD_DIM, EPS, mul, add
                    )
                    y0i = p1s.tile([P, 2 * GT], I32, tag="y0i")
                    nc.vector.tensor_scalar(
                        y0i[:], aa[:].bitcast(I32), 1, None,
                        mybir.AluOpType.logical_shift_right,
                    )
                    nc.vector.tensor_scalar(
                        y0i[:], y0i[:], -1, 0x5F3759DF, mul, add
                    )
                    y0 = y0i[:].bitcast(F32)
                    t1 = p1s.tile([P, 2 * GT], F32, tag="t1")
                    nc.vector.tensor_tensor(t1[:], y0, y0, mul)
                    nc.vector.scalar_tensor_tensor(
                        t1[:], t1[:], -0.5, aa[:], mul, mul
                    )
                    rstd = p1s.tile([P, 2 * GT], F32, tag="rstd")
                    nc.vector.scalar_tensor_tensor(
                        rstd[:], t1[:], 1.5, y0, add, mul
                    )
                    rstdb = p1s.tile([P, 2 * GT], BF16, tag="rstdb")
                    nc.vector.tensor_copy(rstdb[:], rstd[:])
                    tp8 = psp.tile([P, 2, GT, P], BF16, tag="tp8", bufs=1, name="tp8")
                    for u in range(GT):
                        ti = g * GT + u
                        qkv_sb = qkvs[u]
                        csl = csc_sb[:, ti, :]
                        roped = p1s.tile([P, 2, P], BF16, tag="roped")
                        for blk in range(2):
                            xb = qkv_sb[:, blk * P : (blk + 1) * P]
                            m12 = p1s.tile([P, 2, P], BF16, tag="m12")
                            xrep = bass.AP(
                                tensor=xb.tensor,
                                offset=xb.offset,
                                ap=[list(xb.ap[0]), [0, 2], [1, P]],
                            )
                            nc.vector.scalar_tensor_tensor(
                                m12[:],
                                xrep,
                                rstdb[:, 2 * u + blk : 2 * u + blk + 1],
                                csl.rearrange("p (a c) -> p a c", a=2),
                                mul,
                                mul,
                            )
                            mb = m12[:]
                            a_ap = bass.AP(
                                tensor=mb.tensor, offset=mb.offset,
                                ap=[list(mb.ap[0]), [192, 2], [1, 64]],
                            )
                            b_ap = bass.AP(
                                tensor=mb.tensor, offset=mb.offset + 64,
                                ap=[list(mb.ap[0]), [64, 2], [1, 64]],
                            )
                            nc.vector.tensor_tensor(
                                roped[:, blk, :].rearrange(
                                    "p (a c) -> p a c", a=2
                                ),
                                a_ap,
                                b_ap,
                                sub,
                            )
                            nc.tensor.transpose(
                                tp8[:, blk, u, :], roped[:, blk, :], ident[:]
                            )
                        nc.gpsimd.tensor_copy(
                            vv[(b, hl)][:, ti, :], qkv_sb[:, 2 * P : 3 * P]
                        )
                    nc.vector.tensor_copy(
                        qk[(b, hl)][:, :, g * GT : (g + 1) * GT, :], tp8[:]
                    )

                def half_pass(b, hl, weave=()):
                    """Full projection pass for (batch, head): interleave
                    stage_a/stage_b over groups, weaving attention qgroups
                    from `weave` between units."""
                    wl = list(weave)
                    ms = {}
                    qv = {}
                    units = []
                    for g in range(ngrp):
                        units.append(("A", g))
                        if g >= 1:
                            units.append(("B", g - 1))
                    units.append(("B", ngrp - 1))
                    nw = len(wl)
                    for i, (kind, g) in enumerate(units):
                        if kind == "A":
                            ms[g] = p1s.tile(
                                [P, 2 * GT], F32, tag="ms8", bufs=2, name="ms8"
                            )
                            qv[g] = stage_a(b, g, hl, ms[g])
                        else:
                            stage_b(b, g, hl, ms[g], qv[g])
                        if nw:
                            want = (i + 1) * nw // len(units)
                            while len(wl) > nw - want:
                                qgroup(*wl.pop(0))

                def qgroup(b, hl, qg):
                    qkt = qk[(b, hl)]
                    v_t = vv[(b, hl)]
                    qs_ap = qkt[:, 0, qg * gq : (qg + 1) * gq, :]
                    probsT = p2.tile(
                        [P, kc_n, QG], BF16, tag="probsT", bufs=2, name="probsT"
                    )
                    for pr in range(kc_n // 2):
                        sp = psp.tile([P, 2, QG], F32, tag="sp", bufs=2, name="sp")
                        for j in range(2):
                            nc.tensor.matmul(
                                sp[:, j, :],
                                qkt[:, 1, 2 * pr + j, :],
                                qs_ap,
                                start=True,
                                stop=True,
                            )
                        nc.scalar.activation(
                            out=probsT[:, 2 * pr : 2 * pr + 2, :],
                            in_=sp[:],
                            func=mybir.ActivationFunctionType.Exp,
                            scale=SCALE,
                        )
                    av_ps = psp.tile([P, QG], F32, tag="av", bufs=1, name="av_ps")
                    for kc in range(kc_n):
                        nc.tensor.matmul(
                            av_ps[:],
                            v_t[:, kc, :],
                            probsT[:, kc, :],
                            start=(kc == 0),
                            stop=(kc == kc_n - 1),
                        )
                    f1 = p2.tile([P, 8 * QG], BF16, tag="f1", bufs=1)
                    cur = probsT[:].rearrange("p a b -> p (a b)")
                    nc.vector.tensor_tensor(
                        f1[:], cur[:, 0 : 8 * QG], cur[:, 8 * QG : 16 * QG], add
                    )
                    f2 = p2.tile([P, 4 * QG], BF16, tag="f2", bufs=1)
                    nc.vector.tensor_tensor(
                        f2[:], f1[:, 0 : 4 * QG], f1[:, 4 * QG : 8 * QG], add
                    )
                    f3 = p2.tile([P, 2 * QG], BF16, tag="f3", bufs=2)
                    nc.vector.tensor_tensor(
                        f3[:], f2[:, 0 : 2 * QG], f2[:, 2 * QG : 4 * QG], add
                    )
                    nc.sync.dma_start(out=sums_ext[b, hl, qg, :, :], in_=f3[:])
                    av_sb = p2.tile([P, QG], F32, tag="avsb", bufs=2, name="av_sb")
                    nc.vector.tensor_copy(av_sb[:], av_ps[:])
                    nc.sync.dma_start(
                        out=av_ext[b, hl, :, qg * QG : (qg + 1) * QG], in_=av_sb[:]
                    )

                # schedule: 5 windows
                qgs = {
                    (b, hl): [(b, hl, qg) for qg in range(qg_per)]
                    for b in range(batches)
                    for hl in range(HPC)
                }
                half_pass(0, 0)
                half_pass(0, 1, weave=qgs[(0, 0)])
                half_pass(1, 0, weave=qgs[(0, 1)])
                half_pass(1, 1, weave=qgs[(1, 0)])
                for u in qgs[(1, 1)]:
                    qgroup(*u)

    nc.compile()
    return nc


def prep_inputs(x, w_qkv, b_qkv, cos, sin):
    """Build per-core input maps (host-side sharding + retiling)."""
    bf16 = ml_dtypes.bfloat16
    batches, seq, dim = x.shape
    ntb = seq // P
    kc_n = dim // P
    xt = np.ascontiguousarray(
        x.reshape(batches * ntb, P, kc_n, P).transpose(3, 0, 2, 1).astype(bf16)
    )
    cosf = cos.astype(np.float32)
    sinf = sin.astype(np.float32)
    csc = np.concatenate([cosf, sinf, -sinf, cosf], axis=1).astype(bf16)
    csc2 = np.ascontiguousarray(csc.reshape(ntb, P, 2 * P).transpose(1, 0, 2))
    dperm = np.concatenate([np.arange(0, HEAD_DIM, 2), np.arange(1, HEAD_DIM, 2)])
    dnat = np.arange(HEAD_DIM)
    in_maps = []
    for c in range(NCORES):
        idx_parts = []
        for hl in range(HPC):
            h = HPC * c + hl
            idx_parts += [
                h * 384 + dperm * 3 + 0,
                h * 384 + dperm * 3 + 1,
                h * 384 + dnat * 3 + 2,
            ]
        idx = np.concatenate(idx_parts)
        wt = w_qkv[idx, :].T.astype(bf16)  # [DIM, 768]
        wt2 = np.ascontiguousarray(
            wt.reshape(kc_n, P, 2 * 3 * HEAD_DIM).transpose(1, 0, 2)
        )
        bb = np.ascontiguousarray(b_qkv[idx].astype(np.float32)[None, :])
        in_maps.append({"xt": xt, "wt": wt2, "bias": bb, "csc": csc2})
    return in_maps


_CACHED = {}


def _get_nc(seq, batches):
    key = (seq, batches)
    if key not in _CACHED:
        _CACHED[key] = build_nc(seq, batches)
    return _CACHED[key]


def run(x, w_qkv, b_qkv, cos, sin, trace=False):
    from concourse.bass_utils import run_bass_kernel_spmd

    batches, seq, _ = x.shape
    nc = _get_nc(seq, batches)
    in_maps = prep_inputs(x, w_qkv, b_qkv, cos, sin)
    res = run_bass_kernel_spmd(
        nc, in_maps, core_ids=list(range(NCORES)), trace=trace
    )
    parts = []
    for c in range(NCORES):
        av = res.results[c]["av"].astype(np.float32)  # [B, HPC, P, seq]
        sums = res.results[c]["sums"].astype(np.float32)  # [B,HPC,qg,P,2*QG]
        qg_per = seq // QG
        den = sums.reshape(B, HPC, qg_per, P, 2, QG).sum(axis=(3, 4))
        den = den.reshape(B, HPC, seq)
        out_c = av / den[:, :, None, :]  # [B, HPC, P, seq]
        out_c = out_c.transpose(0, 3, 1, 2).reshape(B, seq, HPC * P)
        parts.append(out_c)
    out = np.concatenate(parts, axis=-1)
    return np.ascontiguousarray(out.astype(np.float32)), res


def kernel(x, w_qkv, b_qkv, cos, sin):
    out, _ = run(
        np.asarray(x),
        np.asarray(w_qkv),
        np.asarray(b_qkv),
        np.asarray(cos),
        np.asarray(sin),
        trace=False,
    )
    return out
